# revision 1
# baseline (speedup 1.0000x reference)
"""CoAttention kernel for Trainium2, 8 NeuronCores, pure data parallel.

Math shortcut (exact, from softmax shift-invariance): in the reference,
scores1[b,s,r] = A[b,s] + C[b,r] + const, and softmax is over r, so the
attention weights are independent of s:
    visual_att[b,s,:] = softmax_r(tanh(img[b] @ Wi1) @ wa1[D:])
    att_img_features[b,s,:] = p[b] @ img[b]            (same row for all s)
Likewise stage 2's textual_att is independent of the query index i:
    textual_att[b,i,:] = softmax_j(tanh(text[b] @ Wt2) @ wa2[D:])
    att_text_features[b,i,:] = q[b] @ text[b]          (same row for all i)
Wt1/bt1/Wi2/bi2/wa1[:D]/wa2[:D]/ba1/ba2 cancel exactly.

Each core handles B/8 = 4 batches and outputs the per-batch vectors
u[b] (text) and v[b] (img); the host broadcasts them over S.

The heavy matmuls run in bf16 (measured 222 ns per 128x512 matmul on HW
vs ~300-700 ns for f32r, whose 4-byte weight loads don't hide).  The
host pre-casts inputs to bf16 (halves DMA) and X^T is produced by
DRAM->SBUF DMA xbar transposes (2-byte path), so the PE does no
transposition work for the main operands.  PSUM accumulation is fp32;
softmax bookkeeping is fp32.
"""

import numpy as np
import ml_dtypes

import concourse.bacc as bacc
import concourse.mybir as mybir
import concourse.tile as tile
from concourse.bass_utils import run_bass_kernel_spmd

B, S, R, D = 32, 512, 196, 768
NCORES = 8
BPC = B // NCORES          # batches per core
P = 128
KT = D // P                # 6 contraction tiles
NT = D // P                # 6 output-feature tiles
RPAD = 256                 # img tokens padded to 2 tiles
TTOK = BPC * S             # 2048 text tokens per core
ITOK = BPC * RPAD          # 1024 padded img tokens per core
F32 = mybir.dt.float32
BF16 = mybir.dt.bfloat16
AF = mybir.ActivationFunctionType

_CACHE = {}

# schedule-tuning knobs
CFG = {
    "t2t_bufs": 6,
    "small_bufs": 2,
    "pm_bufs": 2,
    "psm_bufs": 1,
    "exp_dep": False,      # force exps after the last tanh (ACT LUT phasing)
    "reps": 1,             # repeat body in a Tile For_i (slope timing only)
}


def _build():
    nc = bacc.Bacc("TRN2", target_bir_lowering=False, debug=False,
                   num_devices=NCORES)
    text_d = nc.dram_tensor("text", [TTOK, D], BF16, kind="ExternalInput").ap()
    img_d = nc.dram_tensor("img", [ITOK, D], BF16, kind="ExternalInput").ap()
    wi1_d = nc.dram_tensor("Wi1", [D, D], BF16, kind="ExternalInput").ap()
    wt2_d = nc.dram_tensor("Wt2", [D, D], BF16, kind="ExternalInput").ap()
    w1_d = nc.dram_tensor("w1", [D], BF16, kind="ExternalInput").ap()
    w2_d = nc.dram_tensor("w2", [D], BF16, kind="ExternalInput").ap()
    u_d = nc.dram_tensor("u_out", [BPC, D], F32, kind="ExternalOutput").ap()
    v_d = nc.dram_tensor("v_out", [BPC, D], F32, kind="ExternalOutput").ap()

    with tile.TileContext(nc) as tc:
        _emit(tc, text_d, img_d, wi1_d, wt2_d, w1_d, w2_d, u_d, v_d)
    nc.compile()
    return nc


def _emit(tc, text_d, img_d, wi1_d, wt2_d, w1_d, w2_d, u_d, v_d):
    from contextlib import ExitStack, nullcontext

    nc = tc.nc
    with ExitStack() as ctx:
        const = ctx.enter_context(tc.tile_pool(name="const", bufs=1))
        xpool = ctx.enter_context(tc.tile_pool(name="x", bufs=1))
        wpool = ctx.enter_context(tc.tile_pool(name="w", bufs=1))
        tpool = ctx.enter_context(tc.tile_pool(name="t2t", bufs=CFG["t2t_bufs"]))
        spool = ctx.enter_context(tc.tile_pool(name="small", bufs=CFG["small_bufs"]))
        psum_main = ctx.enter_context(
            tc.tile_pool(name="pm", bufs=CFG["pm_bufs"], space="PSUM"))
        psum_sm = ctx.enter_context(
            tc.tile_pool(name="psm", bufs=CFG["psm_bufs"], space="PSUM"))

        loop_cm = (tc.For_i(0, CFG["reps"], 1) if CFG["reps"] > 1
                   else nullcontext())
        with loop_cm:
            _emit_body(tc, ctx, locals())


def _emit_body(tc, ctx, env):
    nc = tc.nc
    (const, xpool, wpool, tpool, spool, psum_main, psum_sm) = (
        env[k] for k in ("const", "xpool", "wpool", "tpool", "spool",
                         "psum_main", "psum_sm"))
    (text_d, img_d, wi1_d, wt2_d, w1_d, w2_d, u_d, v_d) = (
        env[k] for k in ("text_d", "img_d", "wi1_d", "wt2_d", "w1_d", "w2_d",
                         "u_d", "v_d"))

    one1 = const.tile([1, 1], F32)
    nc.gpsimd.memset(one1[:], 1.0)

    # ---- X^T via DRAM->SBUF DMA xbar transposes (2-byte path) ----
    # img first so stage A starts early; the host ships it zero-padded to
    # RPAD tokens per batch, so transposes and natural loads are uniform.
    xt_img = xpool.tile([P, KT, ITOK], BF16)         # (128, 6, 1024)
    for k in range(KT):
        nc.sync.dma_start_transpose(xt_img[:, k, :],
                                    img_d[:, k * P:(k + 1) * P])
    wi1_sb = wpool.tile([P, KT, D], BF16)
    wi1_r = wi1_d.rearrange("(ko p) n -> p ko n", p=P)
    for k in range(KT):
        nc.sync.dma_start(wi1_sb[:, k, :], wi1_r[:, k, :])
    w1col = const.tile([P, NT], BF16)
    nc.sync.dma_start(w1col[:], w1_d.rearrange("(no p) -> p no", p=P))
    img_sb = xpool.tile([P, ITOK // P, D], BF16)     # natural, for the wsum
    nc.sync.dma_start(img_sb[:], img_d.rearrange("(to p) n -> p to n", p=P))

    xt_text = xpool.tile([P, KT, TTOK], BF16)        # (128, 6, 2048)
    for k in range(KT):
        nc.sync.dma_start_transpose(xt_text[:, k, :],
                                    text_d[:, k * P:(k + 1) * P])
    wt2_sb = wpool.tile([P, KT, D], BF16)
    wt2_r = wt2_d.rearrange("(ko p) n -> p ko n", p=P)
    for k in range(KT):
        nc.sync.dma_start(wt2_sb[:, k, :], wt2_r[:, k, :])
    w2col = const.tile([P, NT], BF16)
    nc.sync.dma_start(w2col[:], w2_d.rearrange("(no p) -> p no", p=P))
    text_sb = xpool.tile([P, TTOK // P, D], BF16)    # natural, for the wsum
    nc.sync.dma_start(text_sb[:], text_d.rearrange("(to p) n -> p to n", p=P))

    stages = [
        # (xt, x_nat, base_fn, ntile/batch, tokens, n_valid, W, wcol, out)
        (xt_img, img_sb, lambda b: 2 * b, 2, ITOK, R, wi1_sb, w1col, v_d),
        (xt_text, text_sb, lambda b: 4 * b, 4, TTOK, S, wt2_sb, w2col, u_d),
    ]

    # ---- phase 1: Y^T = (X @ W)^T per 512-token chunk, tanh, d = t2t.w ----
    d_rows = {}
    last_tanh = None
    for si, st in enumerate(stages):
        xt, x_nat, base, ntile, tok, n_valid, W_sb, wcol, out_d = st
        nch = tok // 512
        d_rows[si] = const.tile([1, tok], F32, name=f"drow{si}",
                                tag=f"drow{si}")
        for ch in range(nch):
            sl = slice(512 * ch, 512 * (ch + 1))
            dps = psum_sm.tile([1, 512], F32, tag="d")
            for n in range(NT):
                mp = psum_main.tile([P, 512], F32, tag="pm")
                for k in range(KT):
                    nc.tensor.matmul(
                        mp[:],
                        lhsT=W_sb[:, k, n * P:(n + 1) * P],
                        rhs=xt[:, k, sl],
                        start=(k == 0),
                        stop=(k == KT - 1),
                    )
                t2t = tpool.tile([P, 512], BF16, tag="t2t")
                last_tanh = nc.scalar.activation(t2t[:], mp[:], AF.Tanh)
                nc.tensor.matmul(
                    dps[:],
                    lhsT=wcol[:, n:n + 1],
                    rhs=t2t[:],
                    start=(n == 0),
                    stop=(n == NT - 1),
                )
            nc.vector.tensor_copy(d_rows[si][:1, sl], dps[:1, :])

    # ---- phase 2+3 per batch: softmax (no max-sub: |d| is O(0.3) here, exp
    # cannot overflow; softmax is shift-invariant), q columns, weighted sum --
    for si, st in enumerate(stages):
        xt, x_nat, base, ntile, tok, n_valid, W_sb, wcol, out_d = st
        span = tok // BPC                    # 256 img / 512 text
        for b in range(BPC):
            drow = d_rows[si][:1, span * b: span * b + n_valid]
            qrow = spool.tile([1, span], F32, tag="qrow")
            if n_valid < span:
                nc.vector.memset(qrow[:1, n_valid:], 0.0)
            ssum = spool.tile([1, 1], F32, tag="ssum")
            exp_bi = nc.scalar.activation(qrow[:1, :n_valid], drow,
                                          AF.Exp, accum_out=ssum[:1, :1])
            if CFG["exp_dep"]:
                from concourse.tile_rust import add_dep_helper
                add_dep_helper(exp_bi.ins, last_tanh.ins, sync=False,
                               reason="keep exp after all tanh (ACT LUT)")
            rec = spool.tile([1, 1], F32, tag="rec")
            nc.vector.reciprocal(rec[:], ssum[:])

            qcp = psum_sm.tile([P, ntile], F32, tag="qcol")
            for c in range(ntile):
                nc.tensor.transpose(qcp[:, c:c + 1],
                                    qrow[:1, c * P:(c + 1) * P],
                                    one1[:1, :1])
            qcol = spool.tile([P, ntile], BF16, tag="qcolsb")
            nc.vector.tensor_copy(qcol[:], qcp[:])

            ups = psum_sm.tile([1, D], F32, tag="u")
            for off, sz in ((0, 512), (512, 256)):
                for c in range(ntile):
                    nc.tensor.matmul(
                        ups[:1, off:off + sz],
                        lhsT=qcol[:, c:c + 1],
                        rhs=x_nat[:, base(b) + c, off:off + sz],
                        start=(c == 0),
                        stop=(c == ntile - 1),
                    )
            usb = spool.tile([1, D], F32, tag="usb")
            for off, sz in ((0, 512), (512, 256)):
                nc.scalar.activation(usb[:1, off:off + sz],
                                     ups[:1, off:off + sz],
                                     AF.Copy, scale=rec[:1, :1])
            # SWDGE for the tiny result write-outs: HWDGE output DMAs after
            # the xbar transposes crash the exec unit (xbar-mode transition
            # hazard); gpsimd DMAs avoid the HWDGE queues entirely.
            nc.gpsimd.dma_start(out_d[b:b + 1, :], usb[:1, :])


def _get_nc():
    if "nc" not in _CACHE:
        _CACHE["nc"] = _build()
    return _CACHE["nc"]


def kernel(**inputs):
    bf = ml_dtypes.bfloat16
    text = np.asarray(inputs["text_features"], dtype=np.float32).astype(bf)
    img_raw = np.asarray(inputs["img_features"], dtype=np.float32).astype(bf)
    img = np.zeros((B, RPAD, D), dtype=bf)
    img[:, :R, :] = img_raw
    Wi1 = np.asarray(inputs["Wi1"], dtype=np.float32).astype(bf)
    Wt2 = np.asarray(inputs["Wt2"], dtype=np.float32).astype(bf)
    w1 = np.asarray(inputs["wa1"], dtype=np.float32)[D:].astype(bf)
    w2 = np.asarray(inputs["wa2"], dtype=np.float32)[D:].astype(bf)

    nc = _get_nc()
    in_maps = []
    for c in range(NCORES):
        in_maps.append({
            "text": np.ascontiguousarray(
                text[BPC * c:BPC * (c + 1)].reshape(TTOK, D)),
            "img": np.ascontiguousarray(
                img[BPC * c:BPC * (c + 1)].reshape(ITOK, D)),
            "Wi1": Wi1, "Wt2": Wt2, "w1": w1, "w2": w2,
        })
    res = run_bass_kernel_spmd(nc, in_maps, list(range(NCORES)))
    u = np.concatenate([res.results[c]["u_out"] for c in range(NCORES)], axis=0)
    v = np.concatenate([res.results[c]["v_out"] for c in range(NCORES)], axis=0)
    att_text = np.broadcast_to(u[:, None, :], (B, S, D)).astype(np.float32).copy()
    att_img = np.broadcast_to(v[:, None, :], (B, S, D)).astype(np.float32).copy()
    return att_text, att_img



# revision 11
# speedup vs baseline: 2.8162x; 2.8162x over previous
"""CoAttention kernel for Trainium2, 8 NeuronCores, pure data parallel.

Math shortcut (exact, from softmax shift-invariance): in the reference,
scores1[b,s,r] = A[b,s] + C[b,r] + const, and softmax is over r, so the
attention weights are independent of s:
    visual_att[b,s,:] = softmax_r(tanh(img[b] @ Wi1) @ wa1[D:])
    att_img_features[b,s,:] = p[b] @ img[b]            (same row for all s)
Likewise stage 2's textual_att is independent of the query index i:
    textual_att[b,i,:] = softmax_j(tanh(text[b] @ Wt2) @ wa2[D:])
    att_text_features[b,i,:] = q[b] @ text[b]          (same row for all i)
Wt1/bt1/Wi2/bi2/wa1[:D]/wa2[:D]/ba1/ba2 cancel exactly.

Each core handles B/8 = 4 batches and outputs the per-batch vectors
u[b] (text) and v[b] (img); the host broadcasts them over S.

v2 design:
- The heavy X@W matmuls and the d = w.tanh(XW) dot run in fp8 e4m3 with
  MatmulPerfMode.DoubleRow (256-deep contraction per instruction, half
  the per-row PE cost of bf16).  Weights are pre-scaled by 64 on the
  host so fp8 stays out of the denormal range; the 1/64 is folded into
  the tanh/exp activation scale (exact powers of two).
- X^T is pre-transposed on the host and shipped as fp8 (no DMA xbar
  transposes).  X natural stays bf16 — it feeds the weighted sums that
  produce the outputs directly, where fp8 would eat the error budget.
- The d-scores for all (group, chunk) land on separate PSUM partitions,
  are compacted to a [12, 256] SBUF tile, PE-transposed in two shots,
  and exp'd in ONE activation op ([128, 24] columns = 24 token-tiles).
- Softmax denominators: one stationary-qcol matmul gives per-column
  sums, a tiny [24,48] selector matmul groups them per (batch, dtile)
  output row, one DVE reciprocal gives the scales.
- Weighted sums use X-natural as the matmul *stationary* operand with a
  single q column as the moving operand (1-cycle matmuls), accumulated
  per output column in one PSUM bank; the normalization is applied by
  the final PSUM->SBUF copy via a per-partition activation scale.
"""

import numpy as np
import ml_dtypes

import concourse.bacc as bacc
import concourse.mybir as mybir
import concourse.tile as tile
from concourse.bass_utils import run_bass_kernel_spmd
from concourse.masks import make_identity

B, S, R, D = 32, 512, 196, 768
NCORES = 8
BPC = B // NCORES          # batches per core
P = 128
KT = D // P                # 6 contraction tiles
NT = D // P                # 6 output-feature tiles
KP = KT // 2               # 3 DoubleRow contraction pairs
RPAD = 256                 # img tokens padded to 2 tiles per batch
TTOK = BPC * S             # 2048 text tokens per core
ITOK = BPC * RPAD          # 1024 padded img tokens per core
GRP = 1024                 # tokens per tanh group (2 PSUM banks)
CH = 256                   # tokens per DoubleRow matmul chunk
NCH = GRP // CH            # 4 chunks per group
WS = 64.0                  # host-side weight pre-scale (exact power of 2)
F32 = mybir.dt.float32
BF16 = mybir.dt.bfloat16
FP8 = mybir.dt.float8e4
AF = mybir.ActivationFunctionType
DR = mybir.MatmulPerfMode.DoubleRow

NP_FP8 = ml_dtypes.float8_e4m3
NP_BF16 = ml_dtypes.bfloat16

_CACHE = {}

# schedule-tuning knobs
CFG = {
    "pm_bufs": 2,          # [128,1024] f32 = 2 banks each
    "sm_bufs": 2,          # rotating small psum tiles
}


def _tcol(ct):
    """qcol column index for text token-tile ct (0..15)."""
    return ct


def _icol(ci):
    """qcol column index for img token-tile ci (0..7)."""
    return 16 + ci


def _sel_matrix():
    """[25, 48] f32: sel[p, m] = 1 if qcol column p belongs to output
    column m's batch.  m = b*6+dtile for text (0..23), 24+b*6+dtile img.
    Row 24 pairs with the constant-1 qcol column to subtract the RPAD-R
    zero-pad tokens' exp(0)=1 contributions from each img batch sum."""
    sel = np.zeros((25, 48), np.float32)
    for m in range(24):
        b = m // 6
        for i in range(4):
            sel[_tcol(4 * b + i), m] = 1.0
    for m in range(24, 48):
        b = (m - 24) // 6
        sel[_icol(2 * b), m] = 1.0
        sel[_icol(2 * b + 1), m] = 1.0
        sel[24, m] = -(RPAD - R)
    return sel


def _build():
    nc = bacc.Bacc("TRN2", target_bir_lowering=False, debug=False,
                   num_devices=NCORES)
    dr = {}
    dr["xt_t"] = nc.dram_tensor("xt_t", [P, KT, TTOK], FP8,
                                kind="ExternalInput").ap()
    dr["xt_i"] = nc.dram_tensor("xt_i", [P, KT, ITOK], FP8,
                                kind="ExternalInput").ap()
    dr["nat_t"] = nc.dram_tensor("nat_t", [P, TTOK // P, D], BF16,
                                 kind="ExternalInput").ap()
    dr["nat_i"] = nc.dram_tensor("nat_i", [P, ITOK // P, D], BF16,
                                 kind="ExternalInput").ap()
    dr["w1"] = nc.dram_tensor("w1", [P, KT, D], FP8,
                              kind="ExternalInput").ap()
    dr["w2"] = nc.dram_tensor("w2", [P, KT, D], FP8,
                              kind="ExternalInput").ap()
    dr["wc1"] = nc.dram_tensor("wc1", [P, KT, 1], FP8,
                               kind="ExternalInput").ap()
    dr["wc2"] = nc.dram_tensor("wc2", [P, KT, 1], FP8,
                               kind="ExternalInput").ap()
    dr["sel"] = nc.dram_tensor("sel", [25, 48], F32,
                               kind="ExternalInput").ap()
    dr["u_out"] = nc.dram_tensor("u_out", [BPC, D], F32,
                                 kind="ExternalOutput").ap()
    dr["v_out"] = nc.dram_tensor("v_out", [BPC, D], F32,
                                 kind="ExternalOutput").ap()

    with tile.TileContext(nc) as tc:
        _emit(tc, dr)
    nc.compile()
    return nc


def _emit(tc, dr):
    from contextlib import ExitStack

    nc = tc.nc
    with ExitStack() as ctx:
        const = ctx.enter_context(tc.tile_pool(name="const", bufs=1))
        xpool = ctx.enter_context(tc.tile_pool(name="x", bufs=1))
        spool = ctx.enter_context(tc.tile_pool(name="small", bufs=1))
        pm_ps = ctx.enter_context(
            tc.tile_pool(name="pm", bufs=CFG["pm_bufs"], space="PSUM"))
        sm_ps = ctx.enter_context(
            tc.tile_pool(name="sm", bufs=CFG["sm_bufs"], space="PSUM"))
        _emit_body(tc, const, xpool, spool, pm_ps, sm_ps, dr)


def _emit_body(tc, const, xpool, spool, pm_ps, sm_ps, dr):
    nc = tc.nc

    ident = const.tile([P, P], F32)
    make_identity(nc, ident)
    ones = const.tile([P, 1], BF16)
    nc.gpsimd.memset(ones[:], 1.0)

    # ---- input DMAs (all pre-laid-out by the host; order = need order) ----
    w1 = xpool.tile([P, KT, D], FP8)
    xt_i = xpool.tile([P, KT, ITOK], FP8)
    for kp in range(KP):
        nc.sync.dma_start(w1[:, 2 * kp:2 * kp + 2, :],
                          dr["w1"][:, 2 * kp:2 * kp + 2, :])
        nc.sync.dma_start(xt_i[:, 2 * kp:2 * kp + 2, :],
                          dr["xt_i"][:, 2 * kp:2 * kp + 2, :])
    wc1 = const.tile([P, KT, 1], FP8)
    nc.sync.dma_start(wc1[:], dr["wc1"][:])
    w2 = xpool.tile([P, KT, D], FP8)
    xt_t = xpool.tile([P, KT, TTOK], FP8)
    for kp in range(KP):
        nc.sync.dma_start(w2[:, 2 * kp:2 * kp + 2, :],
                          dr["w2"][:, 2 * kp:2 * kp + 2, :])
        nc.sync.dma_start(xt_t[:, 2 * kp:2 * kp + 2, :],
                          dr["xt_t"][:, 2 * kp:2 * kp + 2, :])
    wc2 = const.tile([P, KT, 1], FP8)
    nc.sync.dma_start(wc2[:], dr["wc2"][:])
    sel_sb = const.tile([25, 48], F32)
    nc.sync.dma_start(sel_sb[:], dr["sel"][:])
    nat_i = xpool.tile([P, ITOK // P, D], BF16)
    nc.sync.dma_start(nat_i[:], dr["nat_i"][:])
    nat_t = xpool.tile([P, TTOK // P, D], BF16)
    nc.sync.dma_start(nat_t[:], dr["nat_t"][:])

    t2t_i = xpool.tile([P, KT, ITOK], FP8)
    t2t_t = xpool.tile([P, KT, TTOK], FP8)

    # ---- phase 1: Y^T = (64W)^T X^T in fp8 DoubleRow, tanh(.)/64 -> fp8,
    # then 64*d columns DIRECTLY transposed: t2t is the stationary operand,
    # the 64w column is the 1-wide moving operand -> out [128 tokens, 1] ----
    dcol = sm_ps.tile([P, 24], F32, tag="sm")
    stages = [
        (xt_i, w1, wc1, t2t_i, ITOK, _icol),    # dcol cols 16..23
        (xt_t, w2, wc2, t2t_t, TTOK, _tcol),    # dcol cols 0..15
    ]
    for xt, W, wc, t2t, tok, colf in stages:
        for g in range(tok // GRP):
            for nt in range(NT):
                pm = pm_ps.tile([P, GRP], F32, tag="pm")
                for ch in range(NCH):
                    for kp in range(KP):
                        nc.tensor.matmul(
                            pm[:, CH * ch:CH * (ch + 1)],
                            lhsT=W[:, 2 * kp:2 * kp + 2, nt * P:(nt + 1) * P],
                            rhs=xt[:, 2 * kp:2 * kp + 2,
                                   g * GRP + CH * ch:g * GRP + CH * (ch + 1)],
                            start=(kp == 0),
                            stop=(kp == KP - 1),
                            perf_mode=DR,
                        )
                nc.scalar.activation(t2t[:, nt, g * GRP:(g + 1) * GRP],
                                     pm[:, :], AF.Tanh, scale=1.0 / WS)
            for tt in range(g * (GRP // P), (g + 1) * (GRP // P)):
                col = colf(tt)
                for kp in range(KP):
                    nc.tensor.matmul(
                        dcol[:, col:col + 1],
                        lhsT=t2t[:, 2 * kp:2 * kp + 2, tt * P:(tt + 1) * P],
                        rhs=wc[:, 2 * kp:2 * kp + 2, :],
                        start=(kp == 0),
                        stop=(kp == KP - 1),
                        perf_mode=DR,
                    )

    # ---- phase 2: ONE exp over all 24 token-tile columns; column 24 is a
    # constant 1/128 so its PE column-sum is exactly 1 (pairs with the sel
    # pad-correction row) ----
    qcol = spool.tile([P, 25], BF16)
    nc.scalar.activation(qcol[:, 0:24], dcol[:, :], AF.Exp, scale=1.0 / WS)
    nc.vector.memset(qcol[:, 24:25], 1.0 / P)

    # per-column sums -> per-(batch,dtile) reciprocals
    scol_ps = sm_ps.tile([P, 1], F32, tag="sm")
    nc.tensor.matmul(scol_ps[0:25, :], lhsT=qcol[:, 0:25], rhs=ones[:, 0:1],
                     start=True, stop=True)
    scol_sb = spool.tile([25, 1], F32)
    nc.vector.tensor_copy(scol_sb[:], scol_ps[0:25, :])
    s48_ps = sm_ps.tile([P, 1], F32, tag="sm")
    nc.tensor.matmul(s48_ps[0:48, :], lhsT=sel_sb[0:25, :],
                     rhs=scol_sb[0:25, :], start=True, stop=True)
    s48_sb = spool.tile([48, 1], F32)
    nc.vector.tensor_copy(s48_sb[:], s48_ps[0:48, :])
    rec48 = spool.tile([48, 1], F32)
    nc.vector.reciprocal(rec48[:], s48_sb[:])

    # ---- phase 3: weighted sums, X natural as stationary, q as moving ----
    upsT = sm_ps.tile([P, 48], F32, tag="sm")
    for m in range(48):
        if m < 24:
            b, dt_ = divmod(m, 6)
            pairs = [(4 * b + i, _tcol(4 * b + i)) for i in range(4)]
            nat = nat_t
        else:
            b, dt_ = divmod(m - 24, 6)
            pairs = [(2 * b + i, _icol(2 * b + i)) for i in range(2)]
            nat = nat_i
        last = len(pairs) - 1
        for i, (tt, col) in enumerate(pairs):
            nc.tensor.matmul(
                upsT[:, m:m + 1],
                lhsT=nat[:, tt, dt_ * P:(dt_ + 1) * P],
                rhs=qcol[:, col:col + 1],
                start=(i == 0),
                stop=(i == last),
            )
    ut_sb = spool.tile([P, 48], F32)
    nc.vector.tensor_copy(ut_sb[:], upsT[:])
    out48 = sm_ps.tile([48, P], F32, tag="sm")
    nc.tensor.transpose(out48[:], ut_sb[:], ident[:, :])
    out_sb = spool.tile([48, P], F32)
    nc.scalar.activation(out_sb[:], out48[0:48, :], AF.Copy,
                         scale=rec48[:, 0:1])

    u_view = dr["u_out"].rearrange("b (t j) -> (b t) j", j=P)
    v_view = dr["v_out"].rearrange("b (t j) -> (b t) j", j=P)
    nc.gpsimd.dma_start(u_view, out_sb[0:24, :])
    nc.gpsimd.dma_start(v_view, out_sb[24:48, :])


def _get_nc():
    if "nc" not in _CACHE:
        _CACHE["nc"] = _build()
    return _CACHE["nc"]


def _prep_inputs(inputs):
    """Full inputs -> list of NCORES per-core input dicts (host layouts)."""
    text = np.asarray(inputs["text_features"], np.float32)
    img_raw = np.asarray(inputs["img_features"], np.float32)
    img = np.zeros((B, RPAD, D), np.float32)
    img[:, :R, :] = img_raw

    Wi1 = np.asarray(inputs["Wi1"], np.float32) * WS
    Wt2 = np.asarray(inputs["Wt2"], np.float32) * WS
    w1v = np.asarray(inputs["wa1"], np.float32)[D:] * WS
    w2v = np.asarray(inputs["wa2"], np.float32)[D:] * WS

    # weights: [P, KT, D] fp8 with W[p, j, n] = 64*W[j*128+p, n]
    w1_l = np.ascontiguousarray(
        Wi1.reshape(KT, P, D).transpose(1, 0, 2)).astype(NP_FP8)
    w2_l = np.ascontiguousarray(
        Wt2.reshape(KT, P, D).transpose(1, 0, 2)).astype(NP_FP8)
    wc1_l = np.ascontiguousarray(
        w1v.reshape(KT, P).T.reshape(P, KT, 1)).astype(NP_FP8)
    wc2_l = np.ascontiguousarray(
        w2v.reshape(KT, P).T.reshape(P, KT, 1)).astype(NP_FP8)
    sel = _sel_matrix()

    in_maps = []
    for c in range(NCORES):
        tc_ = text[BPC * c:BPC * (c + 1)].reshape(TTOK, D)
        ic_ = img[BPC * c:BPC * (c + 1)].reshape(ITOK, D)
        # X^T: [P, KT, TOK] with A[p, j, t] = X[t, j*128+p]
        xt_t = np.ascontiguousarray(
            tc_.T.reshape(KT, P, TTOK).transpose(1, 0, 2)).astype(NP_FP8)
        xt_i = np.ascontiguousarray(
            ic_.T.reshape(KT, P, ITOK).transpose(1, 0, 2)).astype(NP_FP8)
        # X natural: [P, TOK//P, D] with B[p, t, n] = X[t*128+p, n]
        nat_t = np.ascontiguousarray(
            tc_.reshape(TTOK // P, P, D).transpose(1, 0, 2)).astype(NP_BF16)
        nat_i = np.ascontiguousarray(
            ic_.reshape(ITOK // P, P, D).transpose(1, 0, 2)).astype(NP_BF16)
        in_maps.append({
            "xt_t": xt_t, "xt_i": xt_i, "nat_t": nat_t, "nat_i": nat_i,
            "w1": w1_l, "w2": w2_l, "wc1": wc1_l, "wc2": wc2_l, "sel": sel,
        })
    return in_maps


def kernel(**inputs):
    nc = _get_nc()
    in_maps = _prep_inputs(inputs)
    res = run_bass_kernel_spmd(nc, in_maps, list(range(NCORES)))
    u = np.concatenate([res.results[c]["u_out"] for c in range(NCORES)],
                       axis=0)
    v = np.concatenate([res.results[c]["v_out"] for c in range(NCORES)],
                       axis=0)
    att_text = np.broadcast_to(u[:, None, :], (B, S, D)).astype(
        np.float32).copy()
    att_img = np.broadcast_to(v[:, None, :], (B, S, D)).astype(
        np.float32).copy()
    return att_text, att_img


# revision 16
# speedup vs baseline: 2.9243x; 1.0384x over previous
"""CoAttention kernel for Trainium2, 8 NeuronCores, pure data parallel.

Math shortcut (exact, from softmax shift-invariance): in the reference,
scores1[b,s,r] = A[b,s] + C[b,r] + const, and softmax is over r, so the
attention weights are independent of s:
    visual_att[b,s,:] = softmax_r(tanh(img[b] @ Wi1) @ wa1[D:])
    att_img_features[b,s,:] = p[b] @ img[b]            (same row for all s)
Likewise stage 2's textual_att is independent of the query index i:
    textual_att[b,i,:] = softmax_j(tanh(text[b] @ Wt2) @ wa2[D:])
    att_text_features[b,i,:] = q[b] @ text[b]          (same row for all i)
Wt1/bt1/Wi2/bi2/wa1[:D]/wa2[:D]/ba1/ba2 cancel exactly.

Each core handles B/8 = 4 batches and outputs the per-batch vectors
u[b] (text) and v[b] (img); the host broadcasts them over S.

Design:
- The heavy X@W matmuls run in fp8 e4m3 with MatmulPerfMode.DoubleRow
  (256-deep contraction per instruction, half the per-row PE cost of
  bf16).  Weights are pre-scaled by 64 on the host so fp8 stays out of
  the denormal range; the 1/64 is folded into the tanh/exp activation
  scale (exact powers of two).
- X^T is pre-transposed on the host and shipped as fp8 (no DMA xbar
  transposes).  X natural stays bf16 — it feeds the weighted sums that
  produce the outputs directly, where fp8 would eat the error budget.
- d = w.tanh(XW) is computed directly in TRANSPOSED form: t2t (fp8) is
  the matmul stationary operand and the 64w column the 1-wide moving
  operand, so [128 tokens, 1] d columns accumulate straight into a
  PSUM dcol tile (1-cycle matmuls, no copies or transposes), emitted
  eagerly after the tanh of each ntile pair.
- Per stage: ONE exp activation over all token-tile columns; softmax
  denominators via a stationary-qcol column-sum matmul, a tiny
  selector matmul (whose extra constant row exactly subtracts the img
  zero-padding's exp(0)=1 terms), one DVE reciprocal.
- Weighted sums use X-natural as the matmul *stationary* operand with a
  single q column as the moving operand (1-cycle matmuls), accumulated
  per output column in one PSUM bank; normalization is applied by a
  DVE per-partition tensor_scalar multiply after a PE transpose.
- The img stage's finish chain is emitted between the two text groups
  so it overlaps text compute (ACT stays tanh/exp-only and in order);
  outputs go out over HWDGE as two [24, 128] row blocks.
"""

import numpy as np
import ml_dtypes

import concourse.bacc as bacc
import concourse.mybir as mybir
import concourse.tile as tile
from concourse.bass_utils import run_bass_kernel_spmd
from concourse.masks import make_identity

B, S, R, D = 32, 512, 196, 768
NCORES = 8
BPC = B // NCORES          # batches per core
P = 128
KT = D // P                # 6 contraction tiles
NT = D // P                # 6 output-feature tiles
KP = KT // 2               # 3 DoubleRow contraction pairs
RPAD = 256                 # img tokens padded to 2 tiles per batch
TTOK = BPC * S             # 2048 text tokens per core
ITOK = BPC * RPAD          # 1024 padded img tokens per core
GRP = 1024                 # tokens per tanh group (2 PSUM banks)
CH = 256                   # tokens per DoubleRow matmul chunk
NCH = GRP // CH            # 4 chunks per group
WS = 64.0                  # host-side weight pre-scale (exact power of 2)
F32 = mybir.dt.float32
BF16 = mybir.dt.bfloat16
FP8 = mybir.dt.float8e4
AF = mybir.ActivationFunctionType
DR = mybir.MatmulPerfMode.DoubleRow

NP_FP8 = ml_dtypes.float8_e4m3
NP_BF16 = ml_dtypes.bfloat16

_CACHE = {}

# schedule-tuning knobs
CFG = {
    "pm_bufs": 2,          # [128,1024] f32 = 2 banks each
    "sm_bufs": 2,          # rotating small psum tiles
}


def _sel_matrices():
    """Per-stage selector matrices mapping per-token-tile exp sums to the
    24 (batch, dtile) output rows.  sel_i row 8 pairs with the constant
    1/128 qcol column to subtract the RPAD-R zero-pad tokens' exp(0)=1
    contributions from each img batch sum."""
    sel_t = np.zeros((16, 24), np.float32)
    sel_i = np.zeros((9, 24), np.float32)
    for m in range(24):
        b = m // 6
        for i in range(4):
            sel_t[4 * b + i, m] = 1.0
        sel_i[2 * b, m] = 1.0
        sel_i[2 * b + 1, m] = 1.0
        sel_i[8, m] = -(RPAD - R)
    return sel_t, sel_i


def _build():
    nc = bacc.Bacc("TRN2", target_bir_lowering=False, debug=False,
                   num_devices=NCORES)
    dr = {}
    dr["xt_t"] = nc.dram_tensor("xt_t", [P, KT, TTOK], FP8,
                                kind="ExternalInput").ap()
    dr["xt_i"] = nc.dram_tensor("xt_i", [P, KT, ITOK], FP8,
                                kind="ExternalInput").ap()
    dr["nat_t"] = nc.dram_tensor("nat_t", [P, TTOK // P, D], BF16,
                                 kind="ExternalInput").ap()
    dr["nat_i"] = nc.dram_tensor("nat_i", [P, ITOK // P, D], BF16,
                                 kind="ExternalInput").ap()
    dr["w1"] = nc.dram_tensor("w1", [P, KT, D], FP8,
                              kind="ExternalInput").ap()
    dr["w2"] = nc.dram_tensor("w2", [P, KT, D], FP8,
                              kind="ExternalInput").ap()
    dr["wc1"] = nc.dram_tensor("wc1", [P, KT, 1], FP8,
                               kind="ExternalInput").ap()
    dr["wc2"] = nc.dram_tensor("wc2", [P, KT, 1], FP8,
                               kind="ExternalInput").ap()
    dr["sel_t"] = nc.dram_tensor("sel_t", [16, 24], F32,
                                 kind="ExternalInput").ap()
    dr["sel_i"] = nc.dram_tensor("sel_i", [9, 24], F32,
                                 kind="ExternalInput").ap()
    dr["u_out"] = nc.dram_tensor("u_out", [BPC, D], F32,
                                 kind="ExternalOutput").ap()
    dr["v_out"] = nc.dram_tensor("v_out", [BPC, D], F32,
                                 kind="ExternalOutput").ap()

    with tile.TileContext(nc) as tc:
        _emit(tc, dr)
    nc.compile()
    return nc


def _emit(tc, dr):
    from contextlib import ExitStack

    nc = tc.nc
    with ExitStack() as ctx:
        const = ctx.enter_context(tc.tile_pool(name="const", bufs=1))
        xpool = ctx.enter_context(tc.tile_pool(name="x", bufs=1))
        spool = ctx.enter_context(tc.tile_pool(name="small", bufs=1))
        pm_ps = ctx.enter_context(
            tc.tile_pool(name="pm", bufs=CFG["pm_bufs"], space="PSUM"))
        sm_ps = ctx.enter_context(
            tc.tile_pool(name="sm", bufs=CFG["sm_bufs"], space="PSUM"))
        _emit_body(tc, const, xpool, spool, pm_ps, sm_ps, dr)


def _emit_body(tc, const, xpool, spool, pm_ps, sm_ps, dr):
    nc = tc.nc

    ident = const.tile([P, P], F32)
    make_identity(nc, ident)
    ones = const.tile([P, 1], BF16)
    nc.gpsimd.memset(ones[:], 1.0)

    # ---- input DMAs (all pre-laid-out by the host; order = need order) ----
    w1 = xpool.tile([P, KT, D], FP8)
    xt_i = xpool.tile([P, KT, ITOK], FP8)
    for kp in range(KP):
        nc.sync.dma_start(w1[:, 2 * kp:2 * kp + 2, :],
                          dr["w1"][:, 2 * kp:2 * kp + 2, :])
        nc.sync.dma_start(xt_i[:, 2 * kp:2 * kp + 2, :],
                          dr["xt_i"][:, 2 * kp:2 * kp + 2, :])
    wc1 = const.tile([P, KT, 1], FP8)
    nc.sync.dma_start(wc1[:], dr["wc1"][:])
    w2 = xpool.tile([P, KT, D], FP8)
    xt_t = xpool.tile([P, KT, TTOK], FP8)
    for kp in range(KP):
        nc.sync.dma_start(w2[:, 2 * kp:2 * kp + 2, :],
                          dr["w2"][:, 2 * kp:2 * kp + 2, :])
        nc.sync.dma_start(xt_t[:, 2 * kp:2 * kp + 2, :],
                          dr["xt_t"][:, 2 * kp:2 * kp + 2, :])
    wc2 = const.tile([P, KT, 1], FP8)
    nc.sync.dma_start(wc2[:], dr["wc2"][:])
    sel_t = const.tile([16, 24], F32)
    nc.sync.dma_start(sel_t[:], dr["sel_t"][:])
    sel_i = const.tile([9, 24], F32)
    nc.sync.dma_start(sel_i[:], dr["sel_i"][:])
    nat_i = xpool.tile([P, ITOK // P, D], BF16)
    nc.sync.dma_start(nat_i[:], dr["nat_i"][:])
    nat_t = xpool.tile([P, TTOK // P, D], BF16)
    nc.sync.dma_start(nat_t[:], dr["nat_t"][:])

    t2t_i = xpool.tile([P, KT, ITOK], FP8)
    t2t_t = xpool.tile([P, KT, TTOK], FP8)
    dcol_i = sm_ps.tile([P, ITOK // P], F32, tag="dcol")
    dcol_t = sm_ps.tile([P, TTOK // P], F32, tag="dcol")

    def stage_group(xt, W, wc, t2t, dcol, g):
        """One 1024-token group: 6 ntile matmul+tanh rounds, then the
        d-dot columns.  Each dcol column's start..stop accumulation runs
        to completion before the next column's start: a PSUM start marks
        the whole 2KB zero region pending-zero, so interleaving groups
        in one bank corrupts earlier columns' partial sums."""
        for nt in range(NT):
            pm = pm_ps.tile([P, GRP], F32, tag="pm")
            for ch in range(NCH):
                for kp in range(KP):
                    nc.tensor.matmul(
                        pm[:, CH * ch:CH * (ch + 1)],
                        lhsT=W[:, 2 * kp:2 * kp + 2, nt * P:(nt + 1) * P],
                        rhs=xt[:, 2 * kp:2 * kp + 2,
                               g * GRP + CH * ch:g * GRP + CH * (ch + 1)],
                        start=(kp == 0),
                        stop=(kp == KP - 1),
                        perf_mode=DR,
                    )
            nc.scalar.activation(t2t[:, nt, g * GRP:(g + 1) * GRP],
                                 pm[:, :], AF.Tanh, scale=1.0 / WS)
        for tt in range(g * (GRP // P), (g + 1) * (GRP // P)):
            for kp in range(KP):
                nc.tensor.matmul(
                    dcol[:, tt:tt + 1],
                    lhsT=t2t[:, 2 * kp:2 * kp + 2, tt * P:(tt + 1) * P],
                    rhs=wc[:, 2 * kp:2 * kp + 2, :],
                    start=(kp == 0),
                    stop=(kp == KP - 1),
                    perf_mode=DR,
                )

    def stage_exp(dcol, ncol, qcol, pad):
        nc.scalar.activation(qcol[:, 0:ncol], dcol[:, 0:ncol], AF.Exp,
                             scale=1.0 / WS)
        if pad:
            nc.vector.memset(qcol[:, ncol:ncol + 1], 1.0 / P)

    def stage_finish(qcol, nq, sel, nat, tiles_per_b, out_view, tag):
        # softmax denominators: per-column sums -> selector -> reciprocal
        scol_ps = sm_ps.tile([P, 1], F32, tag="sm", name=f"scol{tag}")
        nc.tensor.matmul(scol_ps[0:nq, :], lhsT=qcol[:, 0:nq],
                         rhs=ones[:, 0:1], start=True, stop=True)
        scol_sb = spool.tile([nq, 1], F32, name=f"scolsb{tag}")
        nc.vector.tensor_copy(scol_sb[:], scol_ps[0:nq, :])
        s24_ps = sm_ps.tile([P, 1], F32, tag="sm", name=f"s24{tag}")
        nc.tensor.matmul(s24_ps[0:24, :], lhsT=sel[0:nq, :],
                         rhs=scol_sb[0:nq, :], start=True, stop=True)
        s24_sb = spool.tile([24, 1], F32, name=f"s24sb{tag}")
        nc.vector.tensor_copy(s24_sb[:], s24_ps[0:24, :])
        rec = spool.tile([24, 1], F32, name=f"rec{tag}")
        nc.vector.reciprocal(rec[:], s24_sb[:])
        # weighted sums: X natural stationary, q column moving (1-cycle)
        ups = sm_ps.tile([P, 24], F32, tag="sm", name=f"ups{tag}")
        for m in range(24):
            b, dt_ = divmod(m, 6)
            tts = [tiles_per_b * b + i for i in range(tiles_per_b)]
            for i, tt in enumerate(tts):
                nc.tensor.matmul(
                    ups[:, m:m + 1],
                    lhsT=nat[:, tt, dt_ * P:(dt_ + 1) * P],
                    rhs=qcol[:, tt:tt + 1],
                    start=(i == 0),
                    stop=(i == len(tts) - 1),
                )
        ut_sb = spool.tile([P, 24], F32, name=f"ut{tag}")
        nc.vector.tensor_copy(ut_sb[:], ups[:])
        o24 = sm_ps.tile([24, P], F32, tag="sm", name=f"o24{tag}")
        nc.tensor.transpose(o24[:], ut_sb[:], ident[:, :])
        osb = spool.tile([24, P], F32, name=f"osb{tag}")
        nc.vector.tensor_scalar_mul(osb[:], o24[0:24, :], rec[:, 0:1])
        nc.sync.dma_start(out_view, osb[0:24, :])

    # ---- pipeline: img groups, img exp (ACT stays tanh-paced), text g0,
    # img finish (PE ops land in the xt_t DMA shadow), text g1, text finish.
    u_view = dr["u_out"].rearrange("b (t j) -> (b t) j", j=P)
    v_view = dr["v_out"].rearrange("b (t j) -> (b t) j", j=P)
    qcol_i = spool.tile([P, ITOK // P + 1], BF16)
    qcol_t = spool.tile([P, TTOK // P], BF16)

    stage_group(xt_i, w1, wc1, t2t_i, dcol_i, 0)
    stage_exp(dcol_i, ITOK // P, qcol_i, pad=True)
    stage_group(xt_t, w2, wc2, t2t_t, dcol_t, 0)
    stage_finish(qcol_i, ITOK // P + 1, sel_i, nat_i, 2, v_view, "i")
    stage_group(xt_t, w2, wc2, t2t_t, dcol_t, 1)
    stage_exp(dcol_t, TTOK // P, qcol_t, pad=False)
    stage_finish(qcol_t, TTOK // P, sel_t, nat_t, 4, u_view, "t")


def _get_nc():
    if "nc" not in _CACHE:
        _CACHE["nc"] = _build()
    return _CACHE["nc"]


def _prep_inputs(inputs):
    """Full inputs -> list of NCORES per-core input dicts (host layouts)."""
    text = np.asarray(inputs["text_features"], np.float32)
    img_raw = np.asarray(inputs["img_features"], np.float32)
    img = np.zeros((B, RPAD, D), np.float32)
    img[:, :R, :] = img_raw

    Wi1 = np.asarray(inputs["Wi1"], np.float32) * WS
    Wt2 = np.asarray(inputs["Wt2"], np.float32) * WS
    w1v = np.asarray(inputs["wa1"], np.float32)[D:] * WS
    w2v = np.asarray(inputs["wa2"], np.float32)[D:] * WS

    # weights: [P, KT, D] fp8 with W[p, j, n] = 64*W[j*128+p, n]
    w1_l = np.ascontiguousarray(
        Wi1.reshape(KT, P, D).transpose(1, 0, 2)).astype(NP_FP8)
    w2_l = np.ascontiguousarray(
        Wt2.reshape(KT, P, D).transpose(1, 0, 2)).astype(NP_FP8)
    wc1_l = np.ascontiguousarray(
        w1v.reshape(KT, P).T.reshape(P, KT, 1)).astype(NP_FP8)
    wc2_l = np.ascontiguousarray(
        w2v.reshape(KT, P).T.reshape(P, KT, 1)).astype(NP_FP8)
    sel_t, sel_i = _sel_matrices()

    in_maps = []
    for c in range(NCORES):
        tc_ = text[BPC * c:BPC * (c + 1)].reshape(TTOK, D)
        ic_ = img[BPC * c:BPC * (c + 1)].reshape(ITOK, D)
        # X^T: [P, KT, TOK] with A[p, j, t] = X[t, j*128+p]
        xt_t = np.ascontiguousarray(
            tc_.T.reshape(KT, P, TTOK).transpose(1, 0, 2)).astype(NP_FP8)
        xt_i = np.ascontiguousarray(
            ic_.T.reshape(KT, P, ITOK).transpose(1, 0, 2)).astype(NP_FP8)
        # X natural: [P, TOK//P, D] with B[p, t, n] = X[t*128+p, n]
        nat_t = np.ascontiguousarray(
            tc_.reshape(TTOK // P, P, D).transpose(1, 0, 2)).astype(NP_BF16)
        nat_i = np.ascontiguousarray(
            ic_.reshape(ITOK // P, P, D).transpose(1, 0, 2)).astype(NP_BF16)
        in_maps.append({
            "xt_t": xt_t, "xt_i": xt_i, "nat_t": nat_t, "nat_i": nat_i,
            "w1": w1_l, "w2": w2_l, "wc1": wc1_l, "wc2": wc2_l,
            "sel_t": sel_t, "sel_i": sel_i,
        })
    return in_maps


def kernel(**inputs):
    nc = _get_nc()
    in_maps = _prep_inputs(inputs)
    res = run_bass_kernel_spmd(nc, in_maps, list(range(NCORES)))
    u = np.concatenate([res.results[c]["u_out"] for c in range(NCORES)],
                       axis=0)
    v = np.concatenate([res.results[c]["v_out"] for c in range(NCORES)],
                       axis=0)
    att_text = np.broadcast_to(u[:, None, :], (B, S, D)).astype(
        np.float32).copy()
    att_img = np.broadcast_to(v[:, None, :], (B, S, D)).astype(
        np.float32).copy()
    return att_text, att_img


# revision 17
# speedup vs baseline: 2.9430x; 1.0064x over previous
"""CoAttention kernel for Trainium2, 8 NeuronCores, pure data parallel.

Math shortcut (exact, from softmax shift-invariance): in the reference,
scores1[b,s,r] = A[b,s] + C[b,r] + const, and softmax is over r, so the
attention weights are independent of s:
    visual_att[b,s,:] = softmax_r(tanh(img[b] @ Wi1) @ wa1[D:])
    att_img_features[b,s,:] = p[b] @ img[b]            (same row for all s)
Likewise stage 2's textual_att is independent of the query index i:
    textual_att[b,i,:] = softmax_j(tanh(text[b] @ Wt2) @ wa2[D:])
    att_text_features[b,i,:] = q[b] @ text[b]          (same row for all i)
Wt1/bt1/Wi2/bi2/wa1[:D]/wa2[:D]/ba1/ba2 cancel exactly.

Each core handles B/8 = 4 batches and outputs the per-batch vectors
u[b] (text) and v[b] (img); the host broadcasts them over S.

Design:
- The heavy X@W matmuls run in fp8 e4m3 with MatmulPerfMode.DoubleRow
  (256-deep contraction per instruction, half the per-row PE cost of
  bf16).  Weights are pre-scaled by 64 on the host so fp8 stays out of
  the denormal range; the 1/64 is folded into the tanh/exp activation
  scale (exact powers of two).
- X^T is pre-transposed on the host and shipped as fp8 (no DMA xbar
  transposes).  X natural stays bf16 — it feeds the weighted sums that
  produce the outputs directly, where fp8 would eat the error budget.
- d = w.tanh(XW) is computed directly in TRANSPOSED form: t2t (fp8) is
  the matmul stationary operand and the 64w column the 1-wide moving
  operand, so [128 tokens, 1] d columns accumulate straight into a
  PSUM dcol tile (1-cycle matmuls, no copies or transposes), emitted
  eagerly after the tanh of each ntile pair.
- Per stage: ONE exp activation over all token-tile columns; softmax
  denominators via a stationary-qcol column-sum matmul, a tiny
  selector matmul (whose extra constant row exactly subtracts the img
  zero-padding's exp(0)=1 terms), one DVE reciprocal.
- Weighted sums use X-natural as the matmul *stationary* operand with a
  single q column as the moving operand (1-cycle matmuls), accumulated
  per output column in one PSUM bank; normalization is applied by a
  DVE per-partition tensor_scalar multiply after a PE transpose.
- The img stage's finish chain is emitted between the two text groups
  so it overlaps text compute (ACT stays tanh/exp-only and in order);
  outputs go out over HWDGE as two [24, 128] row blocks.
"""

import numpy as np
import ml_dtypes

import concourse.bacc as bacc
import concourse.mybir as mybir
import concourse.tile as tile
from concourse.bass_utils import run_bass_kernel_spmd
from concourse.masks import make_identity

B, S, R, D = 32, 512, 196, 768
NCORES = 8
BPC = B // NCORES          # batches per core
P = 128
KT = D // P                # 6 contraction tiles
NT = D // P                # 6 output-feature tiles
KP = KT // 2               # 3 DoubleRow contraction pairs
RPAD = 256                 # img tokens padded to 2 tiles per batch
TTOK = BPC * S             # 2048 text tokens per core
ITOK = BPC * RPAD          # 1024 padded img tokens per core
GRP = 1024                 # tokens per tanh group (2 PSUM banks)
CH = 256                   # tokens per DoubleRow matmul chunk
NCH = GRP // CH            # 4 chunks per group
WS = 64.0                  # host-side weight pre-scale (exact power of 2)
F32 = mybir.dt.float32
BF16 = mybir.dt.bfloat16
FP8 = mybir.dt.float8e4
AF = mybir.ActivationFunctionType
DR = mybir.MatmulPerfMode.DoubleRow

NP_FP8 = ml_dtypes.float8_e4m3
NP_BF16 = ml_dtypes.bfloat16

_CACHE = {}

# schedule-tuning knobs
CFG = {
    "pm_bufs": 3,          # [128,1024] f32 = 2 banks each
    "sm_bufs": 1,          # rotating small psum tiles
}


def _sel_matrices():
    """Per-stage selector matrices mapping per-token-tile exp sums to the
    24 (batch, dtile) output rows.  sel_i row 8 pairs with the constant
    1/128 qcol column to subtract the RPAD-R zero-pad tokens' exp(0)=1
    contributions from each img batch sum."""
    sel_t = np.zeros((16, 24), np.float32)
    sel_i = np.zeros((9, 24), np.float32)
    for m in range(24):
        b = m // 6
        for i in range(4):
            sel_t[4 * b + i, m] = 1.0
        sel_i[2 * b, m] = 1.0
        sel_i[2 * b + 1, m] = 1.0
        sel_i[8, m] = -(RPAD - R)
    return sel_t, sel_i


def _build():
    nc = bacc.Bacc("TRN2", target_bir_lowering=False, debug=False,
                   num_devices=NCORES)
    dr = {}
    dr["xt_t"] = nc.dram_tensor("xt_t", [P, KT, TTOK], FP8,
                                kind="ExternalInput").ap()
    dr["xt_i"] = nc.dram_tensor("xt_i", [P, KT, ITOK], FP8,
                                kind="ExternalInput").ap()
    dr["nat_t"] = nc.dram_tensor("nat_t", [P, TTOK // P, D], BF16,
                                 kind="ExternalInput").ap()
    dr["nat_i"] = nc.dram_tensor("nat_i", [P, ITOK // P, D], BF16,
                                 kind="ExternalInput").ap()
    dr["w1"] = nc.dram_tensor("w1", [P, KT, D], FP8,
                              kind="ExternalInput").ap()
    dr["w2"] = nc.dram_tensor("w2", [P, KT, D], FP8,
                              kind="ExternalInput").ap()
    dr["wc1"] = nc.dram_tensor("wc1", [P, KT, 1], FP8,
                               kind="ExternalInput").ap()
    dr["wc2"] = nc.dram_tensor("wc2", [P, KT, 1], FP8,
                               kind="ExternalInput").ap()
    dr["sel_t"] = nc.dram_tensor("sel_t", [16, 24], F32,
                                 kind="ExternalInput").ap()
    dr["sel_i"] = nc.dram_tensor("sel_i", [9, 24], F32,
                                 kind="ExternalInput").ap()
    dr["u_out"] = nc.dram_tensor("u_out", [BPC, D], F32,
                                 kind="ExternalOutput").ap()
    dr["v_out"] = nc.dram_tensor("v_out", [BPC, D], F32,
                                 kind="ExternalOutput").ap()

    with tile.TileContext(nc) as tc:
        _emit(tc, dr)
    nc.compile()
    return nc


def _emit(tc, dr):
    from contextlib import ExitStack

    nc = tc.nc
    with ExitStack() as ctx:
        const = ctx.enter_context(tc.tile_pool(name="const", bufs=1))
        xpool = ctx.enter_context(tc.tile_pool(name="x", bufs=1))
        spool = ctx.enter_context(tc.tile_pool(name="small", bufs=1))
        pm_ps = ctx.enter_context(
            tc.tile_pool(name="pm", bufs=CFG["pm_bufs"], space="PSUM"))
        sm_ps = ctx.enter_context(
            tc.tile_pool(name="sm", bufs=CFG["sm_bufs"], space="PSUM"))
        _emit_body(tc, const, xpool, spool, pm_ps, sm_ps, dr)


def _emit_body(tc, const, xpool, spool, pm_ps, sm_ps, dr):
    nc = tc.nc

    # ---- input DMAs (all pre-laid-out by the host; order = need order;
    # w1/xt_i split by k-pair so the PE can start on partial data) ----
    w1 = xpool.tile([P, KT, D], FP8)
    xt_i = xpool.tile([P, KT, ITOK], FP8)
    for kp in range(KP):
        nc.sync.dma_start(w1[:, 2 * kp:2 * kp + 2, :],
                          dr["w1"][:, 2 * kp:2 * kp + 2, :])
        nc.sync.dma_start(xt_i[:, 2 * kp:2 * kp + 2, :],
                          dr["xt_i"][:, 2 * kp:2 * kp + 2, :])
    wc1 = const.tile([P, KT, 1], FP8)
    nc.sync.dma_start(wc1[:], dr["wc1"][:])
    w2 = xpool.tile([P, KT, D], FP8)
    nc.sync.dma_start(w2[:], dr["w2"][:])
    xt_t = xpool.tile([P, KT, TTOK], FP8)
    nc.sync.dma_start(xt_t[:], dr["xt_t"][:])
    wc2 = const.tile([P, KT, 1], FP8)
    nc.sync.dma_start(wc2[:], dr["wc2"][:])
    ident = const.tile([P, P], F32)
    make_identity(nc, ident)
    ones = const.tile([P, 1], BF16)
    nc.gpsimd.memset(ones[:], 1.0)
    sel_t = const.tile([16, 24], F32)
    nc.sync.dma_start(sel_t[:], dr["sel_t"][:])
    sel_i = const.tile([9, 24], F32)
    nc.sync.dma_start(sel_i[:], dr["sel_i"][:])
    nat_i = xpool.tile([P, ITOK // P, D], BF16)
    nc.sync.dma_start(nat_i[:], dr["nat_i"][:])
    nat_t = xpool.tile([P, TTOK // P, D], BF16)
    nc.sync.dma_start(nat_t[:], dr["nat_t"][:])

    t2t_i = xpool.tile([P, KT, ITOK], FP8)
    t2t_t = xpool.tile([P, KT, TTOK], FP8)
    dcol_all = sm_ps.tile([P, 24], F32, tag="dcol")
    dcol_t = dcol_all[:, 0:16]
    dcol_i = dcol_all[:, 16:24]

    def stage_group(xt, W, wc, t2t, dcol, g):
        """One 1024-token group: 6 ntile matmul+tanh rounds, then the
        d-dot columns.  Each dcol column's start..stop accumulation runs
        to completion before the next column's start: a PSUM start marks
        the whole 2KB zero region pending-zero, so interleaving groups
        in one bank corrupts earlier columns' partial sums."""
        for nt in range(NT):
            pm = pm_ps.tile([P, GRP], F32, tag="pm")
            for ch in range(NCH):
                for kp in range(KP):
                    nc.tensor.matmul(
                        pm[:, CH * ch:CH * (ch + 1)],
                        lhsT=W[:, 2 * kp:2 * kp + 2, nt * P:(nt + 1) * P],
                        rhs=xt[:, 2 * kp:2 * kp + 2,
                               g * GRP + CH * ch:g * GRP + CH * (ch + 1)],
                        start=(kp == 0),
                        stop=(kp == KP - 1),
                        perf_mode=DR,
                    )
            nc.scalar.activation(t2t[:, nt, g * GRP:(g + 1) * GRP],
                                 pm[:, :], AF.Tanh, scale=1.0 / WS)
        for tt in range(g * (GRP // P), (g + 1) * (GRP // P)):
            for kp in range(KP):
                nc.tensor.matmul(
                    dcol[:, tt:tt + 1],
                    lhsT=t2t[:, 2 * kp:2 * kp + 2, tt * P:(tt + 1) * P],
                    rhs=wc[:, 2 * kp:2 * kp + 2, :],
                    start=(kp == 0),
                    stop=(kp == KP - 1),
                    perf_mode=DR,
                )

    def stage_exp(dcol, ncol, qcol, pad):
        nc.scalar.activation(qcol[:, 0:ncol], dcol[:, 0:ncol], AF.Exp,
                             scale=1.0 / WS)
        if pad:
            nc.vector.memset(qcol[:, ncol:ncol + 1], 1.0 / P)

    def stage_finish(qcol, nq, sel, nat, tiles_per_b, out_view, tag):
        # softmax denominators: per-column sums -> selector -> reciprocal
        scol_ps = sm_ps.tile([P, 1], F32, tag="sm", name=f"scol{tag}")
        nc.tensor.matmul(scol_ps[0:nq, :], lhsT=qcol[:, 0:nq],
                         rhs=ones[:, 0:1], start=True, stop=True)
        scol_sb = spool.tile([nq, 1], F32, name=f"scolsb{tag}")
        nc.vector.tensor_copy(scol_sb[:], scol_ps[0:nq, :])
        s24_ps = sm_ps.tile([P, 1], F32, tag="sm", name=f"s24{tag}")
        nc.tensor.matmul(s24_ps[0:24, :], lhsT=sel[0:nq, :],
                         rhs=scol_sb[0:nq, :], start=True, stop=True)
        s24_sb = spool.tile([24, 1], F32, name=f"s24sb{tag}")
        nc.vector.tensor_copy(s24_sb[:], s24_ps[0:24, :])
        rec = spool.tile([24, 1], F32, name=f"rec{tag}")
        nc.vector.reciprocal(rec[:], s24_sb[:])
        # weighted sums: X natural stationary, q column moving (1-cycle)
        ups = sm_ps.tile([P, 24], F32, tag="sm", name=f"ups{tag}")
        for m in range(24):
            b, dt_ = divmod(m, 6)
            tts = [tiles_per_b * b + i for i in range(tiles_per_b)]
            for i, tt in enumerate(tts):
                nc.tensor.matmul(
                    ups[:, m:m + 1],
                    lhsT=nat[:, tt, dt_ * P:(dt_ + 1) * P],
                    rhs=qcol[:, tt:tt + 1],
                    start=(i == 0),
                    stop=(i == len(tts) - 1),
                )
        ut_sb = spool.tile([P, 24], F32, name=f"ut{tag}")
        nc.vector.tensor_copy(ut_sb[:], ups[:])
        o24 = sm_ps.tile([24, P], F32, tag="sm", name=f"o24{tag}")
        nc.tensor.transpose(o24[:], ut_sb[:], ident[:, :])
        osb = spool.tile([24, P], F32, name=f"osb{tag}")
        nc.vector.tensor_scalar_mul(osb[:], o24[0:24, :], rec[:, 0:1])
        nc.sync.dma_start(out_view, osb[0:24, :])

    # ---- pipeline: img groups, img exp (ACT stays tanh-paced), text g0,
    # img finish (PE ops land in the xt_t DMA shadow), text g1, text finish.
    u_view = dr["u_out"].rearrange("b (t j) -> (b t) j", j=P)
    v_view = dr["v_out"].rearrange("b (t j) -> (b t) j", j=P)
    qcol_i = spool.tile([P, ITOK // P + 1], BF16)
    qcol_t = spool.tile([P, TTOK // P], BF16)

    stage_group(xt_i, w1, wc1, t2t_i, dcol_i, 0)
    stage_exp(dcol_i, ITOK // P, qcol_i, pad=True)
    stage_group(xt_t, w2, wc2, t2t_t, dcol_t, 0)
    stage_finish(qcol_i, ITOK // P + 1, sel_i, nat_i, 2, v_view, "i")
    stage_group(xt_t, w2, wc2, t2t_t, dcol_t, 1)
    stage_exp(dcol_t, TTOK // P, qcol_t, pad=False)
    stage_finish(qcol_t, TTOK // P, sel_t, nat_t, 4, u_view, "t")


def _get_nc():
    if "nc" not in _CACHE:
        _CACHE["nc"] = _build()
    return _CACHE["nc"]


def _prep_inputs(inputs):
    """Full inputs -> list of NCORES per-core input dicts (host layouts)."""
    text = np.asarray(inputs["text_features"], np.float32)
    img_raw = np.asarray(inputs["img_features"], np.float32)
    img = np.zeros((B, RPAD, D), np.float32)
    img[:, :R, :] = img_raw

    Wi1 = np.asarray(inputs["Wi1"], np.float32) * WS
    Wt2 = np.asarray(inputs["Wt2"], np.float32) * WS
    w1v = np.asarray(inputs["wa1"], np.float32)[D:] * WS
    w2v = np.asarray(inputs["wa2"], np.float32)[D:] * WS

    # weights: [P, KT, D] fp8 with W[p, j, n] = 64*W[j*128+p, n]
    w1_l = np.ascontiguousarray(
        Wi1.reshape(KT, P, D).transpose(1, 0, 2)).astype(NP_FP8)
    w2_l = np.ascontiguousarray(
        Wt2.reshape(KT, P, D).transpose(1, 0, 2)).astype(NP_FP8)
    wc1_l = np.ascontiguousarray(
        w1v.reshape(KT, P).T.reshape(P, KT, 1)).astype(NP_FP8)
    wc2_l = np.ascontiguousarray(
        w2v.reshape(KT, P).T.reshape(P, KT, 1)).astype(NP_FP8)
    sel_t, sel_i = _sel_matrices()

    in_maps = []
    for c in range(NCORES):
        tc_ = text[BPC * c:BPC * (c + 1)].reshape(TTOK, D)
        ic_ = img[BPC * c:BPC * (c + 1)].reshape(ITOK, D)
        # X^T: [P, KT, TOK] with A[p, j, t] = X[t, j*128+p]
        xt_t = np.ascontiguousarray(
            tc_.T.reshape(KT, P, TTOK).transpose(1, 0, 2)).astype(NP_FP8)
        xt_i = np.ascontiguousarray(
            ic_.T.reshape(KT, P, ITOK).transpose(1, 0, 2)).astype(NP_FP8)
        # X natural: [P, TOK//P, D] with B[p, t, n] = X[t*128+p, n]
        nat_t = np.ascontiguousarray(
            tc_.reshape(TTOK // P, P, D).transpose(1, 0, 2)).astype(NP_BF16)
        nat_i = np.ascontiguousarray(
            ic_.reshape(ITOK // P, P, D).transpose(1, 0, 2)).astype(NP_BF16)
        in_maps.append({
            "xt_t": xt_t, "xt_i": xt_i, "nat_t": nat_t, "nat_i": nat_i,
            "w1": w1_l, "w2": w2_l, "wc1": wc1_l, "wc2": wc2_l,
            "sel_t": sel_t, "sel_i": sel_i,
        })
    return in_maps


def kernel(**inputs):
    nc = _get_nc()
    in_maps = _prep_inputs(inputs)
    res = run_bass_kernel_spmd(nc, in_maps, list(range(NCORES)))
    u = np.concatenate([res.results[c]["u_out"] for c in range(NCORES)],
                       axis=0)
    v = np.concatenate([res.results[c]["v_out"] for c in range(NCORES)],
                       axis=0)
    att_text = np.broadcast_to(u[:, None, :], (B, S, D)).astype(
        np.float32).copy()
    att_img = np.broadcast_to(v[:, None, :], (B, S, D)).astype(
        np.float32).copy()
    return att_text, att_img
